# revision 13
# baseline (speedup 1.0000x reference)
"""DescriptorLoss kernel for Trainium2 (8 NeuronCores, SPMD data-parallel).

Math (d' = 5*d, hinges at d'=1 (neg branch, m=0) and d'=5 (pos branch, m=1)):
    loss*5*N = sum_{m=0} relu(d'-1) + 250 * sum_{m=1} relu(5-d')

Per core: shard = (batch, 16-row i-slab) -> 1024 ij rows x 4096 kl cols,
8 groups (128 rows) x 2 pairs (2048 cols) = 16 pair-tiles.

Identity: with dM = d' - 1024*m (PE-injected mask offset), t1 = dM - 1,
u = |t1|:
  - m=0: u = |d'-1| <= ~510;  m=1: u = 1025-d' in [~515, ~1535]
    (ranges separated; |d'| < 9 sigma ~ 510).
  - hinge1 = sum relu(t1) = 0.5*(sum t1 + sum u); sum t1 is linear
    (rank-1 a.b sums + mask popcount) -> host f64.
  - hinge2 = sum relu(u-1020) = sum u - sum min(u, 1020).

Pipeline per pair [128 x 2048]:
  PE:  4 mains (a5 stationary) + 4 injects (idn=-I stationary, mask 0/1024
       fp8 moving) -> PSUM fp32 dM.
  ACT: activation(Abs, bias=-1) PSUM->SBUF fp16 u-tile + accum_out = sum u.
  DVE: per group, one tensor_scalar (min 1020, add-reduce) FD=4096 over the
       u-tile -> sum min(u, 1020).
Host combines in f64; loss = total / (5*B*IJ^2).
"""

import numpy as np
import ml_dtypes

import concourse.bacc as bacc
import concourse.mybir as mybir
import concourse.tile as tile
from concourse.bass_utils import run_bass_kernel_spmd

B, D, H, W = 2, 128, 64, 64
N_CORES = 8
IJ = H * W               # 4096
ROWS = IJ // 4           # 1024 rows per core
G = ROWS // 128          # 8 row groups
PAIR = 2048              # egress tile width
N_PAIRS = G * 2          # 16
OMEGA = 1024.0
TH = OMEGA - 4.0         # 1020
GPS_MIN_PAIRS = ()       # TensorScalarPtr reduce unsupported on Pool engine
MMW = 512                # matmul moving-operand width (PSUM bank limit)

_cached = {}


def _build_program():
    nc = bacc.Bacc("TRN2")
    f32 = mybir.dt.float32
    bf16 = mybir.dt.bfloat16
    f16 = mybir.dt.float16
    f8 = mybir.dt.float8e5
    Alu = mybir.AluOpType
    Act = mybir.ActivationFunctionType

    a5 = nc.declare_dram_parameter("a5", [D, ROWS], bf16, isOutput=False)
    bm = nc.declare_dram_parameter("bm", [D, IJ], bf16, isOutput=False)
    m8 = nc.declare_dram_parameter("m8", [ROWS, IJ], f8, isOutput=False)
    idn = nc.declare_dram_parameter("idn", [D, D], bf16, isOutput=False)
    accs_out = nc.declare_dram_parameter(
        "accs", [128, 2 * N_PAIRS], f32, isOutput=True)

    with tile.TileContext(nc) as tc:
        with (
            tc.tile_pool(name="desc", bufs=1) as desc_pool,
            tc.tile_pool(name="mask", bufs=4) as mask_pool,
            tc.tile_pool(name="t1", bufs=3) as t1_pool,
            tc.tile_pool(name="junk", bufs=2) as junk_pool,
            tc.tile_pool(name="acc", bufs=1) as acc_pool,
            tc.tile_pool(name="ps", bufs=2, space="PSUM") as ps_pool,
        ):
            a_t = desc_pool.tile([D, ROWS], bf16, tag="a")
            b_t = desc_pool.tile([D, IJ], bf16, tag="b")
            id_t = desc_pool.tile([D, D], bf16, tag="idn")
            bias_t = desc_pool.tile([128, 1], f32, tag="bias")
            prime_t = desc_pool.tile([128, 1], f16, tag="prime")
            accA = acc_pool.tile([128, N_PAIRS], f32, tag="accA")
            accC = acc_pool.tile([128, N_PAIRS], f32, tag="accC")

            nc.gpsimd.memset(bias_t[:], -1.0)
            nc.sync.dma_start(a_t[:, :128], a5[:, :128])
            nc.sync.dma_start(b_t[:, :PAIR], bm[:, :PAIR])
            nc.sync.dma_start(id_t[:], idn[:])
            # Prime the ACT table set (Abs): ~2.7us load overlaps early DMAs.
            nc.scalar.activation(prime_t[:], bias_t[:], Act.Abs,
                                 bias=bias_t[:], scale=1.0)

            m_tiles = {}

            def load_mask(g, p):
                mt = mask_pool.tile([128, PAIR], f8, tag="m8")
                rs = slice(g * 128, (g + 1) * 128)
                ks = slice(p * PAIR, (p + 1) * PAIR)
                nc.gpsimd.dma_start(mt[:], m8[rs, ks])
                m_tiles[(g, p)] = mt

            load_mask(0, 0)
            load_mask(0, 1)

            min_queue = []  # (pid, tile, colslice) pending min-ops

            def pair_min(pid, pt, csl):
                jk = junk_pool.tile([128, PAIR], f16, tag="junk")
                eng = nc.gpsimd if pid in GPS_MIN_PAIRS else nc.vector
                eng.tensor_scalar(
                    jk[:], pt[:, csl], TH, 0.0,
                    op0=Alu.min, op1=Alu.add,
                    accum_out=accC[:, pid:pid + 1],
                )

            for g in range(G):
                rs = slice(g * 128, (g + 1) * 128)
                t1_t = t1_pool.tile([128, IJ], f16, tag="t1")

                ps_tiles = {}
                # mains: a-block stationary across the whole group
                for p in range(2):
                    pst = ps_pool.tile([128, PAIR], f32, tag="d")
                    ps_tiles[p] = pst
                    if g == 0 and p == 1:
                        # remaining descriptors, before anything consumes them
                        nc.sync.dma_start(a_t[:, 128:], a5[:, 128:])
                        nc.sync.dma_start(b_t[:, PAIR:], bm[:, PAIR:])
                    for h in range(PAIR // MMW):
                        hs = slice(h * MMW, (h + 1) * MMW)
                        cs = slice(p * PAIR + h * MMW, p * PAIR + (h + 1) * MMW)
                        nc.tensor.matmul(pst[:, hs], a_t[:, rs], b_t[:, cs],
                                         start=True, stop=False)

                if g + 1 < G:
                    load_mask(g + 1, 0)
                    load_mask(g + 1, 1)

                # mask injection, idn stationary across the whole group
                for p in range(2):
                    mt = m_tiles[(g, p)]
                    pst = ps_tiles[p]
                    for h in range(PAIR // MMW):
                        hs = slice(h * MMW, (h + 1) * MMW)
                        nc.tensor.matmul(pst[:, hs], id_t[:], mt[:, hs],
                                         start=False, stop=True)

                # ACT egress per pair: u = |dM - 1| + accum(sum u)
                for p in range(2):
                    pid = g * 2 + p
                    nc.scalar.activation(
                        t1_t[:, p * PAIR:(p + 1) * PAIR], ps_tiles[p][:],
                        Act.Abs, bias=bias_t[:], scale=1.0,
                        accum_out=accA[:, pid:pid + 1],
                    )
                    min_queue.append(
                        (pid, t1_t, slice(p * PAIR, (p + 1) * PAIR)))

                # min-ops lag one pair behind the egress stream
                while len(min_queue) > 1:
                    pair_min(*min_queue.pop(0))

            while min_queue:
                pair_min(*min_queue.pop(0))

            nc.sync.dma_start(accs_out[:, :N_PAIRS], accA[:])
            nc.sync.dma_start(accs_out[:, N_PAIRS:], accC[:])

    nc.finalize()
    return nc


def _prep_inputs(descriptors_0, descriptors_1, similarity_mask):
    d0 = np.asarray(descriptors_0, dtype=np.float32)
    d1 = np.asarray(descriptors_1, dtype=np.float32)
    mkv = np.asarray(similarity_mask)
    idn = (-np.eye(D, dtype=np.float32)).astype(ml_dtypes.bfloat16)
    in_maps = []
    side = []
    for c in range(N_CORES):
        b = c >> 2
        isl = (c & 3) * 16
        a5 = (d0[b].reshape(D, IJ)[:, isl * W:(isl + 16) * W]
              * np.float32(5.0)).astype(ml_dtypes.bfloat16)
        bmv = d1[b].reshape(D, IJ).astype(ml_dtypes.bfloat16)
        mblk = mkv[b, isl:isl + 16].reshape(ROWS, IJ)
        m8v = (mblk.astype(np.float32) * np.float32(OMEGA)).astype(
            ml_dtypes.float8_e5m2)
        in_maps.append(
            {
                "a5": np.ascontiguousarray(a5),
                "bm": np.ascontiguousarray(bmv),
                "m8": np.ascontiguousarray(m8v),
                "idn": np.ascontiguousarray(idn),
            }
        )
        # linear term sum(t1) over the whole shard, f64 from the same
        # bf16 values the PE consumes
        asum = a5.astype(np.float64).sum(axis=1)
        bsum = bmv.astype(np.float64).sum(axis=1)
        s_dp = float(asum @ bsum)
        n1 = float(mblk.sum(dtype=np.int64))
        s_lin = s_dp - OMEGA * n1 - float(ROWS * IJ)
        side.append(s_lin)
    _cached["side"] = side
    return in_maps


def _run(in_maps, **kwargs):
    if "nc" not in _cached:
        _cached["nc"] = _build_program()
    return run_bass_kernel_spmd(_cached["nc"], in_maps, list(range(N_CORES)),
                                **kwargs)


def _combine(results):
    side = _cached["side"]
    total = 0.0
    for r, s_lin in zip(results, side):
        acc = r["accs"].astype(np.float64)
        accA = acc[:, :N_PAIRS]
        accC = acc[:, N_PAIRS:]
        a_tot = accA.sum()
        hinge1 = 0.5 * (s_lin + a_tot)
        hinge2 = 0.0
        for pid in range(N_PAIRS):
            hinge2 += accA[:, pid].sum() - accC[:, pid].sum()
        total += hinge1 + 250.0 * hinge2
    return np.float32(total / (5.0 * B * IJ * IJ))


def kernel(descriptors_0, descriptors_1, similarity_mask):
    in_maps = _prep_inputs(descriptors_0, descriptors_1, similarity_mask)
    res = _run(in_maps)
    return _combine(res.results)


# revision 14
# speedup vs baseline: 1.0453x; 1.0453x over previous
"""DescriptorLoss kernel for Trainium2 (8 NeuronCores, SPMD data-parallel).

Math (d' = 5*d, hinges at d'=1 (neg branch, m=0) and d'=5 (pos branch, m=1)):
    loss*5*N = sum_{m=0} relu(d'-1) + 250 * sum_{m=1} relu(5-d')

Per core: shard = (batch, 16-row i-slab) -> 1024 ij rows x 4096 kl cols,
8 groups (128 rows) x 2 pairs (2048 cols) = 16 pair-tiles.

Identity: with dM = d' - 1024*m (PE-injected mask offset), t1 = dM - 1,
u = |t1|:
  - m=0: u = |d'-1| <= ~510;  m=1: u = 1025-d' in [~515, ~1535]
    (ranges separated; |d'| < 9 sigma ~ 510).
  - hinge1 = sum relu(t1) = 0.5*(sum t1 + sum u); sum t1 is linear
    (rank-1 a.b sums + mask popcount) -> host f64.
  - hinge2 = sum relu(u-1020) = sum u - sum min(u, 1020).

Pipeline per pair [128 x 2048]:
  PE:  4 mains (a5 stationary) + 4 injects (idn=-I stationary, mask 0/1024
       fp8 moving) -> PSUM fp32 dM.
  ACT: activation(Abs, bias=-1) PSUM->SBUF fp16 u-tile + accum_out = sum u.
  DVE: per group, one tensor_scalar (min 1020, add-reduce) FD=4096 over the
       u-tile -> sum min(u, 1020).
Host combines in f64; loss = total / (5*B*IJ^2).
"""

import numpy as np
import ml_dtypes

import concourse.bacc as bacc
import concourse.mybir as mybir
import concourse.tile as tile
from concourse.bass_utils import run_bass_kernel_spmd

B, D, H, W = 2, 128, 64, 64
N_CORES = 8
IJ = H * W               # 4096
ROWS = IJ // 4           # 1024 rows per core
G = ROWS // 128          # 8 row groups
PAIR = 2048              # egress tile width
N_PAIRS = G * 2          # 16
OMEGA = 1024.0
TH = OMEGA - 4.0         # 1020
GPS_MIN_PAIRS = ()       # TensorScalarPtr reduce unsupported on Pool engine
MMW = 512                # matmul moving-operand width (PSUM bank limit)

_cached = {}


def _build_program():
    nc = bacc.Bacc("TRN2")
    f32 = mybir.dt.float32
    bf16 = mybir.dt.bfloat16
    f16 = mybir.dt.float16
    f8 = mybir.dt.float8e5
    Alu = mybir.AluOpType
    Act = mybir.ActivationFunctionType

    a5 = nc.declare_dram_parameter("a5", [D, ROWS], bf16, isOutput=False)
    bm = nc.declare_dram_parameter("bm", [D, IJ], bf16, isOutput=False)
    m8 = nc.declare_dram_parameter("m8", [ROWS, IJ], f8, isOutput=False)
    idn = nc.declare_dram_parameter("idn", [D, D], bf16, isOutput=False)
    accs_out = nc.declare_dram_parameter(
        "accs", [128, 2 * N_PAIRS], f32, isOutput=True)

    with tile.TileContext(nc) as tc:
        with (
            tc.tile_pool(name="desc", bufs=1) as desc_pool,
            tc.tile_pool(name="mask", bufs=4) as mask_pool,
            tc.tile_pool(name="t1", bufs=3) as t1_pool,
            tc.tile_pool(name="junk", bufs=2) as junk_pool,
            tc.tile_pool(name="acc", bufs=1) as acc_pool,
            tc.tile_pool(name="ps", bufs=2, space="PSUM") as ps_pool,
        ):
            a_t = desc_pool.tile([D, ROWS], bf16, tag="a")
            b_t = desc_pool.tile([D, IJ], bf16, tag="b")
            id_t = desc_pool.tile([D, D], bf16, tag="idn")
            bias_t = desc_pool.tile([128, 1], f32, tag="bias")
            prime_t = desc_pool.tile([128, 1], f16, tag="prime")
            accA = acc_pool.tile([128, N_PAIRS], f32, tag="accA")
            accC = acc_pool.tile([128, N_PAIRS], f32, tag="accC")

            nc.gpsimd.memset(bias_t[:], -1.0)
            # Prime the ACT table set (Abs): ~2.7us load overlaps early DMAs.
            nc.scalar.activation(prime_t[:], bias_t[:], Act.Abs,
                                 bias=bias_t[:], scale=1.0)
            # Fine-sliced loads ordered so group-0 work starts ASAP.
            nc.sync.dma_start(a_t[:, :128], a5[:, :128])
            nc.sync.dma_start(b_t[:, :512], bm[:, :512])
            nc.sync.dma_start(id_t[:], idn[:])

            m_tiles = {}

            def load_mask(g, p):
                mt = mask_pool.tile([128, PAIR], f8, tag="m8")
                rs = slice(g * 128, (g + 1) * 128)
                base = p * PAIR
                for h in range(2):
                    nc.sync.dma_start(
                        mt[:, h * 1024:(h + 1) * 1024],
                        m8[rs, base + h * 1024:base + (h + 1) * 1024])
                m_tiles[(g, p)] = mt

            load_mask(0, 0)
            for h in range(3):
                sl = slice(512 + h * 512, 1024 + h * 512)
                nc.sync.dma_start(b_t[:, sl], bm[:, sl])
            load_mask(0, 1)

            min_queue = []  # (pid, tile, colslice) pending min-ops

            def pair_min(pid, pt, csl):
                jk = junk_pool.tile([128, PAIR], f16, tag="junk")
                eng = nc.gpsimd if pid in GPS_MIN_PAIRS else nc.vector
                eng.tensor_scalar(
                    jk[:], pt[:, csl], TH, 0.0,
                    op0=Alu.min, op1=Alu.add,
                    accum_out=accC[:, pid:pid + 1],
                )

            for g in range(G):
                rs = slice(g * 128, (g + 1) * 128)
                t1_t = t1_pool.tile([128, IJ], f16, tag="t1")

                ps_tiles = {}
                # mains: a-block stationary across the whole group
                for p in range(2):
                    pst = ps_pool.tile([128, PAIR], f32, tag="d")
                    ps_tiles[p] = pst
                    if g == 0 and p == 1:
                        # remaining descriptors, before anything consumes them
                        for h in range(4):
                            sl = slice(PAIR + h * 512, PAIR + (h + 1) * 512)
                            nc.sync.dma_start(b_t[:, sl], bm[:, sl])
                        nc.sync.dma_start(a_t[:, 128:], a5[:, 128:])
                    for h in range(PAIR // MMW):
                        hs = slice(h * MMW, (h + 1) * MMW)
                        cs = slice(p * PAIR + h * MMW, p * PAIR + (h + 1) * MMW)
                        nc.tensor.matmul(pst[:, hs], a_t[:, rs], b_t[:, cs],
                                         start=True, stop=False)

                if g + 1 < G:
                    load_mask(g + 1, 0)
                    load_mask(g + 1, 1)

                # mask injection, idn stationary across the whole group
                for p in range(2):
                    mt = m_tiles[(g, p)]
                    pst = ps_tiles[p]
                    for h in range(PAIR // MMW):
                        hs = slice(h * MMW, (h + 1) * MMW)
                        nc.tensor.matmul(pst[:, hs], id_t[:], mt[:, hs],
                                         start=False, stop=True)

                # ACT egress per pair: u = |dM - 1| + accum(sum u)
                for p in range(2):
                    pid = g * 2 + p
                    nc.scalar.activation(
                        t1_t[:, p * PAIR:(p + 1) * PAIR], ps_tiles[p][:],
                        Act.Abs, bias=bias_t[:], scale=1.0,
                        accum_out=accA[:, pid:pid + 1],
                    )
                    min_queue.append(
                        (pid, t1_t, slice(p * PAIR, (p + 1) * PAIR)))

                # min-ops lag one pair behind the egress stream
                while len(min_queue) > 1:
                    pair_min(*min_queue.pop(0))

            while min_queue:
                pair_min(*min_queue.pop(0))

            nc.sync.dma_start(accs_out[:, :N_PAIRS], accA[:])
            nc.sync.dma_start(accs_out[:, N_PAIRS:], accC[:])

    nc.finalize()
    return nc


def _prep_inputs(descriptors_0, descriptors_1, similarity_mask):
    d0 = np.asarray(descriptors_0, dtype=np.float32)
    d1 = np.asarray(descriptors_1, dtype=np.float32)
    mkv = np.asarray(similarity_mask)
    idn = (-np.eye(D, dtype=np.float32)).astype(ml_dtypes.bfloat16)
    in_maps = []
    side = []
    for c in range(N_CORES):
        b = c >> 2
        isl = (c & 3) * 16
        a5 = (d0[b].reshape(D, IJ)[:, isl * W:(isl + 16) * W]
              * np.float32(5.0)).astype(ml_dtypes.bfloat16)
        bmv = d1[b].reshape(D, IJ).astype(ml_dtypes.bfloat16)
        mblk = mkv[b, isl:isl + 16].reshape(ROWS, IJ)
        m8v = (mblk.astype(np.float32) * np.float32(OMEGA)).astype(
            ml_dtypes.float8_e5m2)
        in_maps.append(
            {
                "a5": np.ascontiguousarray(a5),
                "bm": np.ascontiguousarray(bmv),
                "m8": np.ascontiguousarray(m8v),
                "idn": np.ascontiguousarray(idn),
            }
        )
        # linear term sum(t1) over the whole shard, f64 from the same
        # bf16 values the PE consumes
        asum = a5.astype(np.float64).sum(axis=1)
        bsum = bmv.astype(np.float64).sum(axis=1)
        s_dp = float(asum @ bsum)
        n1 = float(mblk.sum(dtype=np.int64))
        s_lin = s_dp - OMEGA * n1 - float(ROWS * IJ)
        side.append(s_lin)
    _cached["side"] = side
    return in_maps


def _run(in_maps, **kwargs):
    if "nc" not in _cached:
        _cached["nc"] = _build_program()
    return run_bass_kernel_spmd(_cached["nc"], in_maps, list(range(N_CORES)),
                                **kwargs)


def _combine(results):
    side = _cached["side"]
    total = 0.0
    for r, s_lin in zip(results, side):
        acc = r["accs"].astype(np.float64)
        accA = acc[:, :N_PAIRS]
        accC = acc[:, N_PAIRS:]
        a_tot = accA.sum()
        hinge1 = 0.5 * (s_lin + a_tot)
        hinge2 = 0.0
        for pid in range(N_PAIRS):
            hinge2 += accA[:, pid].sum() - accC[:, pid].sum()
        total += hinge1 + 250.0 * hinge2
    return np.float32(total / (5.0 * B * IJ * IJ))


def kernel(descriptors_0, descriptors_1, similarity_mask):
    in_maps = _prep_inputs(descriptors_0, descriptors_1, similarity_mask)
    res = _run(in_maps)
    return _combine(res.results)


# revision 15
# speedup vs baseline: 1.0508x; 1.0053x over previous
"""DescriptorLoss kernel for Trainium2 (8 NeuronCores, SPMD data-parallel).

Math (d' = 5*d, hinges at d'=1 (neg branch, m=0) and d'=5 (pos branch, m=1)):
    loss*5*N = sum_{m=0} relu(d'-1) + 250 * sum_{m=1} relu(5-d')

Per core: shard = (batch, 16-row i-slab) -> 1024 ij rows x 4096 kl cols,
8 groups (128 rows) x 2 pairs (2048 cols) = 16 pair-tiles.

Identity: with dM = d' - 1024*m (PE-injected mask offset), t1 = dM - 1,
u = |t1|:
  - m=0: u = |d'-1| <= ~510;  m=1: u = 1025-d' in [~515, ~1535]
    (ranges separated; |d'| < 9 sigma ~ 510).
  - hinge1 = sum relu(t1) = 0.5*(sum t1 + sum u); sum t1 is linear
    (rank-1 a.b sums + mask popcount) -> host f64.
  - hinge2 = sum relu(u-1020) = sum u - sum min(u, 1020).

Pipeline per pair [128 x 2048]:
  PE:  4 mains (a5 stationary) + 4 injects (idn=-I stationary, mask 0/1024
       fp8 moving) -> PSUM fp32 dM.
  ACT: activation(Abs, bias=-1) PSUM->SBUF fp16 u-tile + accum_out = sum u.
  DVE: per group, one tensor_scalar (min 1020, add-reduce) FD=4096 over the
       u-tile -> sum min(u, 1020).
Host combines in f64; loss = total / (5*B*IJ^2).
"""

import numpy as np
import ml_dtypes

import concourse.bacc as bacc
import concourse.mybir as mybir
import concourse.tile as tile
from concourse.bass_utils import run_bass_kernel_spmd

B, D, H, W = 2, 128, 64, 64
N_CORES = 8
IJ = H * W               # 4096
ROWS = IJ // 4           # 1024 rows per core
G = ROWS // 128          # 8 row groups
PAIR = 2048              # egress tile width
N_PAIRS = G * 2          # 16
OMEGA = 1024.0
TH = OMEGA - 4.0         # 1020
GPS_MIN_PAIRS = ()       # TensorScalarPtr reduce unsupported on Pool engine
MMW = 512                # matmul moving-operand width (PSUM bank limit)

_cached = {}


def _build_program():
    nc = bacc.Bacc("TRN2")
    f32 = mybir.dt.float32
    bf16 = mybir.dt.bfloat16
    f16 = mybir.dt.float16
    f8 = mybir.dt.float8e5
    Alu = mybir.AluOpType
    Act = mybir.ActivationFunctionType

    a5 = nc.declare_dram_parameter("a5", [D, ROWS], bf16, isOutput=False)
    bm = nc.declare_dram_parameter("bm", [D, IJ], bf16, isOutput=False)
    m8 = nc.declare_dram_parameter("m8", [ROWS, IJ], f8, isOutput=False)
    idn = nc.declare_dram_parameter("idn", [D, D], bf16, isOutput=False)
    accs_out = nc.declare_dram_parameter(
        "accs", [128, 2 * N_PAIRS], f32, isOutput=True)

    with tile.TileContext(nc) as tc:
        with (
            tc.tile_pool(name="desc", bufs=1) as desc_pool,
            tc.tile_pool(name="mask", bufs=4) as mask_pool,
            tc.tile_pool(name="t1", bufs=3) as t1_pool,
            tc.tile_pool(name="junk", bufs=2) as junk_pool,
            tc.tile_pool(name="acc", bufs=1) as acc_pool,
            tc.tile_pool(name="ps", bufs=2, space="PSUM") as ps_pool,
        ):
            a_t = desc_pool.tile([D, ROWS], bf16, tag="a")
            b_t = desc_pool.tile([D, IJ], bf16, tag="b")
            id_t = desc_pool.tile([D, D], bf16, tag="idn")
            bias_t = desc_pool.tile([128, 1], f32, tag="bias")
            prime_t = desc_pool.tile([128, 1], f16, tag="prime")
            accAll = acc_pool.tile([128, 2 * N_PAIRS], f32, tag="accAll")

            nc.gpsimd.memset(bias_t[:], -1.0)
            # Prime the ACT table set (Abs): ~2.7us load overlaps early DMAs.
            nc.scalar.activation(prime_t[:], bias_t[:], Act.Abs,
                                 bias=bias_t[:], scale=1.0)
            # Fine-sliced loads ordered so group-0 work starts ASAP.
            nc.sync.dma_start(a_t[:, :128], a5[:, :128])
            nc.sync.dma_start(b_t[:, :512], bm[:, :512])
            nc.sync.dma_start(id_t[:], idn[:])

            m_tiles = {}

            def load_mask_group(g, eng):
                mt = mask_pool.tile([128, IJ], f8, tag="m8")
                rs = slice(g * 128, (g + 1) * 128)
                if g == 0:
                    eng.dma_start(mt[:, :PAIR], m8[rs, :PAIR])
                    eng.dma_start(mt[:, PAIR:], m8[rs, PAIR:])
                else:
                    eng.dma_start(mt[:], m8[rs, :])
                m_tiles[g] = mt

            nc.sync.dma_start(b_t[:, 512:PAIR], bm[:, 512:PAIR])
            nc.sync.dma_start(b_t[:, PAIR:], bm[:, PAIR:])
            nc.sync.dma_start(a_t[:, 128:], a5[:, 128:])
            load_mask_group(0, nc.scalar)
            load_mask_group(1, nc.scalar)

            min_queue = []  # (pid, tile, colslice) pending min-ops

            def pair_min(pid, pt, csl):
                jk = junk_pool.tile([128, PAIR], f16, tag="junk")
                eng = nc.gpsimd if pid in GPS_MIN_PAIRS else nc.vector
                eng.tensor_scalar(
                    jk[:], pt[:, csl], TH, 0.0,
                    op0=Alu.min, op1=Alu.add,
                    accum_out=accAll[:, N_PAIRS + pid:N_PAIRS + pid + 1],
                )

            for g in range(G):
                rs = slice(g * 128, (g + 1) * 128)
                t1_t = t1_pool.tile([128, IJ], f16, tag="t1")

                ps_tiles = {}
                # mains: a-block stationary across the whole group
                for p in range(2):
                    pst = ps_pool.tile([128, PAIR], f32, tag="d")
                    ps_tiles[p] = pst

                    for h in range(PAIR // MMW):
                        hs = slice(h * MMW, (h + 1) * MMW)
                        cs = slice(p * PAIR + h * MMW, p * PAIR + (h + 1) * MMW)
                        nc.tensor.matmul(pst[:, hs], a_t[:, rs], b_t[:, cs],
                                         start=True, stop=False)

                if g + 2 < G:
                    load_mask_group(g + 2, nc.sync)

                # mask injection, idn stationary across the whole group
                for p in range(2):
                    mt = m_tiles[g]
                    pst = ps_tiles[p]
                    for h in range(PAIR // MMW):
                        hs = slice(h * MMW, (h + 1) * MMW)
                        nc.tensor.matmul(pst[:, hs],
                                         id_t[:], mt[:, p * PAIR:][:, hs],
                                         start=False, stop=True)

                # ACT egress per pair: u = |dM - 1| + accum(sum u)
                for p in range(2):
                    pid = g * 2 + p
                    nc.scalar.activation(
                        t1_t[:, p * PAIR:(p + 1) * PAIR], ps_tiles[p][:],
                        Act.Abs, bias=bias_t[:], scale=1.0,
                        accum_out=accAll[:, pid:pid + 1],
                    )
                    min_queue.append(
                        (pid, t1_t, slice(p * PAIR, (p + 1) * PAIR)))

                # min-ops lag one pair behind the egress stream
                while len(min_queue) > 1:
                    pair_min(*min_queue.pop(0))

            while min_queue:
                pair_min(*min_queue.pop(0))

            nc.sync.dma_start(accs_out[:], accAll[:])

    nc.finalize()
    return nc


def _prep_inputs(descriptors_0, descriptors_1, similarity_mask):
    d0 = np.asarray(descriptors_0, dtype=np.float32)
    d1 = np.asarray(descriptors_1, dtype=np.float32)
    mkv = np.asarray(similarity_mask)
    idn = (-np.eye(D, dtype=np.float32)).astype(ml_dtypes.bfloat16)
    in_maps = []
    side = []
    for c in range(N_CORES):
        b = c >> 2
        isl = (c & 3) * 16
        a5 = (d0[b].reshape(D, IJ)[:, isl * W:(isl + 16) * W]
              * np.float32(5.0)).astype(ml_dtypes.bfloat16)
        bmv = d1[b].reshape(D, IJ).astype(ml_dtypes.bfloat16)
        mblk = mkv[b, isl:isl + 16].reshape(ROWS, IJ)
        m8v = (mblk.astype(np.float32) * np.float32(OMEGA)).astype(
            ml_dtypes.float8_e5m2)
        in_maps.append(
            {
                "a5": np.ascontiguousarray(a5),
                "bm": np.ascontiguousarray(bmv),
                "m8": np.ascontiguousarray(m8v),
                "idn": np.ascontiguousarray(idn),
            }
        )
        # linear term sum(t1) over the whole shard, f64 from the same
        # bf16 values the PE consumes
        asum = a5.astype(np.float64).sum(axis=1)
        bsum = bmv.astype(np.float64).sum(axis=1)
        s_dp = float(asum @ bsum)
        n1 = float(mblk.sum(dtype=np.int64))
        s_lin = s_dp - OMEGA * n1 - float(ROWS * IJ)
        side.append(s_lin)
    _cached["side"] = side
    return in_maps


def _run(in_maps, **kwargs):
    if "nc" not in _cached:
        _cached["nc"] = _build_program()
    return run_bass_kernel_spmd(_cached["nc"], in_maps, list(range(N_CORES)),
                                **kwargs)


def _combine(results):
    side = _cached["side"]
    total = 0.0
    for r, s_lin in zip(results, side):
        acc = r["accs"].astype(np.float64)
        accA = acc[:, :N_PAIRS]
        accC = acc[:, N_PAIRS:]
        a_tot = accA.sum()
        hinge1 = 0.5 * (s_lin + a_tot)
        hinge2 = 0.0
        for pid in range(N_PAIRS):
            hinge2 += accA[:, pid].sum() - accC[:, pid].sum()
        total += hinge1 + 250.0 * hinge2
    return np.float32(total / (5.0 * B * IJ * IJ))


def kernel(descriptors_0, descriptors_1, similarity_mask):
    in_maps = _prep_inputs(descriptors_0, descriptors_1, similarity_mask)
    res = _run(in_maps)
    return _combine(res.results)
